# revision 10
# baseline (speedup 1.0000x reference)
"""Trainium2 Bass kernel for nn_CausalSelfAttention (BitLinear QKV/O + RoPE + causal attn).

Sharding: 2 heads x 2 batches per core (head-parallel). Per core:
- q/k projections in fp8e4 DoubleRow (2 k-tiles per instruction) with the RoPE
  rotation folded into a second projection weight set (q_rot = (Wq x) o cos +
  ((R Wq) x) o sin), producing fp8 q/k directly from the RoPE add.
- v projection transposed (out = [token, dim]) in bf16 so V lands directly in
  the PV lhsT layout, no PE transposes.
- causal flash-style attention in [k, q] score layout: fp8 DoubleRow score
  matmuls (zero second slot), exp on ACT over packed multi-bank PSUM,
  unnormalized softmax with a ones column on V for the denominator, bf16 PV.
- column-sharded bf16 output projection producing a bf16 partial [4096, 1024]
  summed across cores on the host.
Batch-1 projections are interleaved into batch-0 attention to keep all
engines busy.
"""
import sys

sys.path.insert(0, "/opt/trn_rl_repo")

import numpy as np
import ml_dtypes

F8 = ml_dtypes.float8_e4m3
BF = ml_dtypes.bfloat16

GROUP = 128
N_HEADS = 16
EPS = 1e-8
B, T, C = 2, 2048, 1024
HD = 64
N_CORES = 8
HPC = N_HEADS // N_CORES  # 2 heads per core


# ---------------------------------------------------------------- host prep
def _ternary_quantize(w):
    O, I = w.shape
    g = w.reshape(O, I // GROUP, GROUP).astype(np.float32)
    scale = np.maximum(np.mean(np.abs(g), axis=-1, keepdims=True), EPS).astype(
        np.float32
    )
    wn = g / scale
    q = np.where(wn > 0.5, 1.0, np.where(wn < -0.5, -1.0, 0.0)).astype(np.float32)
    return (q * scale).reshape(O, I).astype(np.float32)


def _rot_rows(wr):
    """RoPE rotate-half on weight rows: per 64-row head block,
    out[0:32] = -w[32:64], out[32:64] = w[0:32]."""
    out = np.empty_like(wr)
    for h in range(wr.shape[0] // HD):
        blk = wr[h * HD : (h + 1) * HD]
        out[h * HD : h * HD + 32] = -blk[32:64]
        out[h * HD + 32 : h * HD + 64] = blk[0:32]
    return out


def _w8_lhsT(w_rows):
    """[128 rows(d), 1024 in] -> DR lhsT [128 p(cin), 4 jj, 2 slot, 128 d] fp8."""
    wsT = w_rows.T  # [1024 in, 128 d]
    t = wsT.reshape(8, 128, 128).transpose(1, 0, 2)  # [p, kt, d]
    t = t.reshape(128, 4, 2, 128)
    return np.ascontiguousarray(t).astype(F8)


def _make_core_inputs(x, wq, wk, wv, wo, rope_cos, rope_sin):
    x = np.ascontiguousarray(x.astype(np.float32).reshape(B * T, C))
    wq_q = _ternary_quantize(wq)
    wk_q = _ternary_quantize(wk)
    wv_q = _ternary_quantize(wv)
    wo_q = _ternary_quantize(wo)

    xT = x.T  # [1024 c, 4096 t]
    xs = xT.reshape(8, 128, 8, 512).transpose(2, 1, 0, 3)  # [s, p, kt, u]
    xb_slab = np.ascontiguousarray(xs).astype(BF)
    x8_slab = np.ascontiguousarray(xs.reshape(8, 128, 4, 2, 512)).astype(F8)

    cosT = rope_cos.astype(np.float32).T  # [32, 2048]
    sinT = rope_sin.astype(np.float32).T
    cos_t = np.tile(cosT, (4, 1)).astype(BF)  # [128, 2048]
    sin_t = np.tile(sinT, (4, 1)).astype(BF)
    tri = (np.arange(128)[None, :] >= np.arange(128)[:, None]).astype(BF)
    ones64 = np.ones((1, 64), np.float32)
    vinit = np.ones((128, HPC, 2, 16, 65), BF)

    maps = []
    for core in range(N_CORES):
        r0 = core * HPC * HD
        rows = slice(r0, r0 + HPC * HD)
        wqr = wq_q[rows, :]
        wkr = wk_q[rows, :]
        woc = wo_q[:, rows]  # [1024 o, 128 d]
        maps.append(
            {
                "xb": xb_slab,
                "x8": x8_slab,
                "wqc": _w8_lhsT(wqr),
                "wqs": _w8_lhsT(_rot_rows(wqr)),
                "wkc": _w8_lhsT(wkr),
                "wks": _w8_lhsT(_rot_rows(wkr)),
                "wv": np.ascontiguousarray(
                    wv_q[rows, :].T.reshape(8, 128, 128).transpose(1, 0, 2)
                ).astype(BF),
                "woC": np.ascontiguousarray(woc.T).astype(BF),  # [128 d, 1024 o]
                "cos_t": cos_t,
                "sin_t": sin_t,
                "tri": tri,
                "ones64": ones64,
                "vinit": vinit,
            }
        )
    return maps


# ---------------------------------------------------------------- BIR post-pass
def _split_excess_waits(nc, max_waits=1):
    """walrus CoreV3 codegen rejects instructions with >1 sem wait; split the
    excess into preceding NoOps on the same engine."""
    import concourse.mybir as mybir

    for f in nc.m.functions:
        for bb in f.blocks:
            insts = bb.instructions
            i = 0
            while i < len(insts):
                ins = insts[i]
                si = ins.sync_info
                if si is not None and si.on_wait and len(si.on_wait) > max_waits:
                    waits = list(si.on_wait)
                    si.on_wait = waits[:max_waits]
                    rest = waits[max_waits:]
                    new_ops = []
                    for j in range(0, len(rest), max_waits):
                        new_ops.append(
                            mybir.InstNoOp(
                                name=nc.get_next_instruction_name(),
                                sync_info=mybir.SyncInfo(
                                    on_wait=rest[j : j + max_waits], on_update=[]
                                ),
                                bass_nofuse=True,
                                engine=ins.engine,
                            )
                        )
                    insts[i:i] = new_ops
                    i += len(new_ops)
                i += 1


# ---------------------------------------------------------------- device kernel
def _emit(nc, tc, d):
    import concourse.mybir as mybir
    from concourse.bass import ds, ts

    f32 = mybir.dt.float32
    f32r = mybir.dt.float32r
    bf16 = mybir.dt.bfloat16
    f8 = mybir.dt.float8e4
    AF = mybir.ActivationFunctionType
    OP = mybir.AluOpType
    DR = mybir.MatmulPerfMode.DoubleRow

    with nc.allow_low_precision(reason="fp8/bf16 matmuls; fp32 accum in PSUM"), \
         tc.tile_pool(name="const", bufs=1) as cp, \
         tc.tile_pool(name="persist", bufs=1) as pp, \
         tc.tile_pool(name="xbP", bufs=4) as xbp, \
         tc.tile_pool(name="x8P", bufs=6) as x8p, \
         tc.tile_pool(name="ropeT", bufs=1) as rtp, \
         tc.tile_pool(name="EP", bufs=2) as epool, \
         tc.tile_pool(name="rcP", bufs=2) as rcp, \
         tc.tile_pool(name="obP", bufs=4) as obp, \
         tc.tile_pool(name="spP", bufs=2, space="PSUM") as spp, \
         tc.tile_pool(name="ypP", bufs=1, space="PSUM") as ypp, \
         tc.tile_pool(name="auxP", bufs=2, space="PSUM") as auxp:

        xb_t = {}
        x8_t = {}
        # small weight/const DMAs first so the first projections aren't
        # queued behind slab transfers
        w8 = {}
        for nm in ("wqc", "wqs", "wkc", "wks"):
            w8[nm] = cp.tile([128, 4, 2, 128], f8, name=nm, tag=nm)
            nc.sync.dma_start(w8[nm][:], d[nm])
        cos_sb = cp.tile([128, 2048], bf16)
        nc.sync.dma_start(cos_sb[:], d["cos_t"])
        sin_sb = cp.tile([128, 2048], bf16)
        nc.sync.dma_start(sin_sb[:], d["sin_t"])
        tri_t = cp.tile([128, 128], bf16)
        nc.sync.dma_start(tri_t[:], d["tri"])
        ones64 = cp.tile([1, 64], f32r)
        nc.sync.dma_start(ones64[:], d["ones64"])
        for s in range(4):
            x8_t[s] = x8p.tile([128, 4, 2, 512], f8, name=f"x8_{s}", tag="x8")
            nc.sync.dma_start(x8_t[s][:], d["x8"][s])

        # persistent activations
        q8 = pp.tile([128, 4096], f8)
        k8z = pp.tile([128, 2, 4096], f8)  # slot 1 stays zero for DR scores
        nc.gpsimd.memset(k8z[:, 1, :], 0.0)
        v_sb = pp.tile([128, 2, 2, 16, 65], bf16)  # [p, h, b, j, 64+ones]
        y2 = pp.tile([128, 4096], bf16)
        y2B = pp.tile([64, 4096], bf16)

        for s in range(4):
            xb_t[s] = xbp.tile([128, 8, 512], bf16, name=f"xb_{s}", tag="xb")
            nc.sync.dma_start(xb_t[s][:], d["xb"][s])
        wv_t = cp.tile([128, 8, 128], bf16)
        nc.sync.dma_start(wv_t[:], d["wv"])
        woC = cp.tile([128, 1024], bf16)
        nc.sync.dma_start(woC[:], d["woC"])
        nc.sync.dma_start(v_sb[:], d["vinit"])
        for s in range(4, 8):
            x8_t[s] = x8p.tile([128, 4, 2, 512], f8, name=f"x8_{s}", tag="x8")
            nc.sync.dma_start(x8_t[s][:], d["x8"][s])
            xb_t[s] = xbp.tile([128, 8, 512], bf16, name=f"xb_{s}", tag="xb")
            nc.sync.dma_start(xb_t[s][:], d["xb"][s])

        # ---------------- Phase A pieces (emitted in chunks) ----------------
        pending = []

        def drain(n):
            for _ in range(min(n, len(pending))):
                pending.pop(0)()

        def drain_all():
            drain(len(pending))

        def emit_qk(H, kinds, halves=(0, 1)):
            """kinds: list of (weight_name, table) pairs; rope add per half."""
            tmp = {}
            for half in halves:
                hcols = ds(H * 2048 + half * 1024, 1024)
                for kind, tbl in kinds:
                    pk = spp.tile(
                        [128, 1024], f32, name=f"pk{kind}{H}{half}", tag="sp"
                    )
                    for sl2 in range(2):
                        sl = half * 2 + sl2
                        for jj in range(4):
                            nc.tensor.matmul(
                                pk[:, ds(sl2 * 512, 512)],
                                w8[kind][:, jj],
                                x8_t[4 * H + sl][:, jj],
                                start=(jj == 0),
                                stop=(jj == 3),
                                perf_mode=DR,
                                skip_group_check=True,
                            )
                    tmp[kind] = rtp.tile(
                        [128, 1024], bf16, name=f"t{kind}{H}{half}", tag=f"t{kind}"
                    )
                    nc.vector.tensor_tensor(
                        tmp[kind][:], pk[:], tbl[:, ds(half * 1024, 1024)], OP.mult
                    )
                    drain(2)
                for cnm, snm, dstt in (
                    ("wqc", "wqs", q8),
                    ("wkc", "wks", k8z),
                ):
                    if cnm in tmp:
                        dst = (
                            q8[:, hcols] if cnm == "wqc" else k8z[:, 0, hcols]
                        )
                        nc.vector.tensor_tensor(
                            dst, tmp[cnm][:], tmp[snm][:], OP.add
                        )

        def emit_v(H, slabs):
            for sl in slabs:
                pv = spp.tile([128, 8, 128], f32, name=f"pv{H}{sl}", tag="sp")
                for tb in range(4):
                    for kt in range(8):
                        nc.tensor.matmul(
                            pv[:, tb, :],
                            xb_t[4 * H + sl][:, kt, ds(tb * 128, 128)],
                            wv_t[:, kt, :],
                            start=(kt == 0),
                            stop=(kt == 7),
                            skip_group_check=True,
                        )
                for h in range(2):
                    nc.vector.tensor_copy(
                        v_sb[:, h, H, ds(sl * 4, 4), 0:64],
                        pv[:, 0:4, ds(64 * h, 64)],
                    )
                drain(2)

        # ---------------- Phase B+C per (b, qi) ----------------
        def emit_attn(b, qi):
            nj = 4 * qi + 4
            qcols = ds(b * 2048 + qi * 512, 512)
            Ns = [512 - max(128 * (j - 4 * qi), 0) for j in range(nj)]
            offs = [0] * nj
            for j in range(1, nj):
                offs[j] = offs[j - 1] + Ns[j - 1]
            E_t = [
                epool.tile([128, 7424], bf16, name=f"E{h}_{b}_{qi}", tag=f"E{h}")
                for h in range(2)
            ]
            yp = [
                ypp.tile([65, 512], f32, name=f"yp{h}_{b}_{qi}", tag=f"yp{h}")
                for h in range(2)
            ]

            def mk_pv(h, j):
                def go():
                    dlt0 = 512 - Ns[j]
                    nc.tensor.matmul(
                        yp[h][:, ds(dlt0, Ns[j])],
                        v_sb[:, h, b, j, :],
                        E_t[h][:, ds(offs[j], Ns[j])],
                        start=(j == 0),
                        stop=(j == nj - 1),
                        skip_group_check=True,
                    )
                return go

            def mk_norm(h):
                def go():
                    rc = rcp.tile([1, 512], f32r, name=f"rc{h}", tag="rc")
                    nc.vector.reciprocal(rc[:], yp[h][64:65, :])
                    rb = auxp.tile([64, 512], f32, name=f"rb{h}", tag="aux")
                    nc.tensor.matmul(rb[:], ones64[:], rc[:], start=True, stop=True)
                    dst = y2[0:64, qcols] if h == 0 else y2B[:, qcols]
                    nc.vector.tensor_tensor(dst, yp[h][0:64, :], rb[:], OP.mult)
                    if h == 1:
                        nc.sync.dma_start(y2[64:128, qcols], y2B[:, qcols])
                return go

            for h in range(2):
                hrow = ds(64 * h, 64)
                for jp in range(nj // 2):
                    j0, j1 = 2 * jp, 2 * jp + 1
                    w01 = Ns[j0] + Ns[j1]
                    sp = spp.tile([128, 1024], f32, name=f"sp{jp}", tag="sp")
                    for j in (j0, j1):
                        qsub = ds(b * 2048 + qi * 512 + (512 - Ns[j]), Ns[j])
                        q8d = (
                            q8[hrow, qsub]
                            .unsqueeze(1)
                            .broadcast_to([64, 2, Ns[j]])
                        )
                        nc.tensor.matmul(
                            sp[:, ds(offs[j] - offs[j0], Ns[j])],
                            k8z[hrow, :, ds(b * 2048 + j * 128, 128)],
                            q8d,
                            start=True,
                            stop=True,
                            perf_mode=DR,
                            skip_group_check=True,
                        )
                    nc.scalar.activation(
                        E_t[h][:, ds(offs[j0], w01)],
                        sp[:, 0:w01],
                        AF.Exp,
                        scale=0.125,
                    )
                    drain(3)
                for dg in range(4):
                    j = 4 * qi + dg
                    nc.gpsimd.tensor_tensor(
                        E_t[h][:, ds(offs[j], 128)],
                        E_t[h][:, ds(offs[j], 128)],
                        tri_t[:],
                        OP.mult,
                    )
                pending.extend(mk_pv(h, j) for j in range(nj))
                pending.append(mk_norm(h))

        def emit_out(b, qi):
            def mk_chunk(tck, oc):
                def go():
                    op = auxp.tile([128, 512], f32, name="op", tag="aux")
                    nc.tensor.matmul(
                        op[:],
                        y2[:, ts(tck, 128)],
                        woC[:, ts(oc, 512)],
                        start=True,
                        stop=True,
                    )
                    ob = obp.tile([128, 512], bf16, name="ob", tag="ob")
                    if (tck * 2 + oc) % 16 < 7:
                        nc.scalar.copy(ob[:], op[:])
                    else:
                        nc.vector.tensor_copy(ob[:], op[:])
                    nc.sync.dma_start(
                        d["outp"][ds(tck * 128, 128), ds(oc * 512, 512)], ob[:]
                    )
                return go

            for tc4 in range(4):
                tck = (b * 4 + qi) * 4 + tc4
                for oc in range(2):
                    pending.append(mk_chunk(tck, oc))

        # ---------------- emission schedule ----------------
        QK = [("wqc", cos_sb), ("wqs", sin_sb), ("wkc", cos_sb), ("wks", sin_sb)]
        emit_qk(0, QK, (0,))
        emit_qk(0, QK, (1,))
        emit_v(0, [0])
        emit_attn(0, 0)
        emit_v(0, [1])
        emit_qk(1, QK[:2], (0,))
        emit_attn(0, 1)
        emit_out(0, 0)
        emit_v(0, [2])
        emit_qk(1, QK[2:], (0,))
        emit_attn(0, 2)
        emit_out(0, 1)
        emit_v(0, [3])
        emit_qk(1, QK[:2], (1,))
        emit_attn(0, 3)
        emit_out(0, 2)
        emit_v(1, [0])
        emit_qk(1, QK[2:], (1,))
        emit_attn(1, 0)
        emit_out(0, 3)
        emit_v(1, [1])
        emit_attn(1, 1)
        emit_out(1, 0)
        emit_v(1, [2])
        emit_attn(1, 2)
        emit_out(1, 1)
        emit_v(1, [3])
        emit_out(1, 2)
        emit_attn(1, 3)
        emit_out(1, 3)
        drain_all()


_NC_CACHE = {}


def _build():
    if "nc" in _NC_CACHE:
        return _NC_CACHE["nc"]
    import concourse.bass as bass
    import concourse.mybir as mybir
    import concourse.tile as tile

    f32 = mybir.dt.float32
    f32r = mybir.dt.float32r
    bf16 = mybir.dt.bfloat16
    f8 = mybir.dt.float8e4
    nc = bass.Bass("TRN2", target_bir_lowering=False, debug=False, num_devices=1)
    d = {
        "xb": nc.dram_tensor("xb", [8, 128, 8, 512], bf16, kind="ExternalInput").ap(),
        "x8": nc.dram_tensor("x8", [8, 128, 4, 2, 512], f8, kind="ExternalInput").ap(),
        "wqc": nc.dram_tensor("wqc", [128, 4, 2, 128], f8, kind="ExternalInput").ap(),
        "wqs": nc.dram_tensor("wqs", [128, 4, 2, 128], f8, kind="ExternalInput").ap(),
        "wkc": nc.dram_tensor("wkc", [128, 4, 2, 128], f8, kind="ExternalInput").ap(),
        "wks": nc.dram_tensor("wks", [128, 4, 2, 128], f8, kind="ExternalInput").ap(),
        "wv": nc.dram_tensor("wv", [128, 8, 128], bf16, kind="ExternalInput").ap(),
        "woC": nc.dram_tensor("woC", [128, 1024], bf16, kind="ExternalInput").ap(),
        "cos_t": nc.dram_tensor("cos_t", [128, 2048], bf16, kind="ExternalInput").ap(),
        "sin_t": nc.dram_tensor("sin_t", [128, 2048], bf16, kind="ExternalInput").ap(),
        "tri": nc.dram_tensor("tri", [128, 128], bf16, kind="ExternalInput").ap(),
        "ones64": nc.dram_tensor("ones64", [1, 64], f32r, kind="ExternalInput").ap(),
        "vinit": nc.dram_tensor(
            "vinit", [128, HPC, 2, 16, 65], bf16, kind="ExternalInput"
        ).ap(),
        "outp": nc.dram_tensor("outp", [4096, 1024], bf16, kind="ExternalOutput").ap(),
    }
    with tile.TileContext(nc) as tc:
        _emit(nc, tc, d)
    _split_excess_waits(nc)
    _NC_CACHE["nc"] = nc
    return nc


def kernel(x, wq, wk, wv, wo, rope_cos, rope_sin):
    from concourse import bass_utils

    x, wq, wk, wv, wo, rope_cos, rope_sin = (
        np.asarray(a, dtype=np.float32)
        for a in (x, wq, wk, wv, wo, rope_cos, rope_sin)
    )
    in_maps = _make_core_inputs(x, wq, wk, wv, wo, rope_cos, rope_sin)
    nc = _build()
    res = bass_utils.run_bass_kernel_spmd(nc, in_maps, core_ids=list(range(N_CORES)))
    total = np.zeros((B * T, C), np.float32)
    for i in range(N_CORES):
        total += np.asarray(res.results[i]["outp"]).astype(np.float32)
    return total.reshape(B, T, C).astype(np.float32)
